# revision 11
# baseline (speedup 1.0000x reference)
"""CRF loss kernel for Trainium2 (8 NeuronCores).

Forward CRF recursion in linear space:
    alpha_t = (alpha_{t-1} @ expT) * exp(o_t)
Positive matrices forget initial conditions geometrically (Birkhoff
contraction; here expT entries are within e^{+-0.5} of 1, so the rate is
~5x per step). The sequence is therefore cut into C short chunks; each
chunk's chain is started from a uniform state W steps early (burn-in) so
its state direction converges to the true alpha before the chunk begins.
Per-group log-growth over the chunk is exact, directions are accurate to
~kappa^W; the scalar log-partition is stitched on the host in fp64.

Device layout: state V[128 labels (partitions), G groups (free dim)].
One matmul with the constant expT as stationary weights advances ALL G
groups one step; a DVE elementwise multiply applies the per-position
emission factors (host-gathered tiles). Emissions are pre-scaled by
exp(-mu) (mu = mean per-step log growth) so no renormalisation is ever
needed: fp32 state drift stays within e^{+-3}.

Group 0 has no real burn-in data (clamped), so its chunk is recomputed
exactly on the host in fp64 and its device result discarded.
"""

import os
import numpy as np

SEQ = 131072
L = 126          # real labels; transitions is (L+2, L+2) = (128, 128)
NL = 128
N_CORES = 8

# ---- tunables -------------------------------------------------------------
S = 2            # superchains (PSUM banks) per core
G = 512          # groups per superchain (one matmul/DVE op advances G chains)
W = 16           # burn-in steps
E_DT = "float32"  # emission tile dtype on device: "float32" or "bfloat16"
# ---------------------------------------------------------------------------

C = N_CORES * S * G               # total groups
CHUNK = SEQ // C                  # steps per group
T = W + CHUNK                     # device steps per superchain
assert CHUNK * C == SEQ

last_exec_time_ns = None


def _build_program():
    import concourse.bacc as bacc
    import concourse.mybir as mybir
    from concourse.tile import TileContext

    e_dt = getattr(mybir.dt, E_DT)
    f32 = mybir.dt.float32

    nc = bacc.Bacc("TRN2", target_bir_lowering=False, debug=False)
    w_d = nc.dram_tensor("w", [NL, NL], f32, kind="ExternalInput")
    e_ds = [
        nc.dram_tensor(f"e{s}", [NL, T * G], e_dt, kind="ExternalInput")
        for s in range(S)
    ]
    snap_ds = [
        nc.dram_tensor(f"snap{s}", [NL, G], f32, kind="ExternalOutput")
        for s in range(S)
    ]
    fin_ds = [
        nc.dram_tensor(f"fin{s}", [NL, G], f32, kind="ExternalOutput")
        for s in range(S)
    ]

    with TileContext(nc) as tc:
        with tc.tile_pool(name="wp", bufs=1) as wp, \
             tc.tile_pool(name="ep", bufs=6) as ep, \
             tc.tile_pool(name="vp", bufs=3) as vp, \
             tc.tile_pool(name="pp", bufs=2, space="PSUM") as pp:
            # Stage weights through a DVE copy: the first matmul then waits
            # only on the DVE semaphore (PE LDW allows a single sync wait).
            w_stage = wp.tile([NL, NL], f32, tag="wstage")
            nc.sync.dma_start(w_stage[:], w_d[:])
            w_t = wp.tile([NL, NL], f32, tag="wt")
            nc.vector.tensor_copy(w_t[:], w_stage[:])
            v_cur = []
            for s in range(S):
                v0 = vp.tile([NL, G], f32, tag=f"v{s}")
                nc.vector.memset(v0[:], 1.0)
                v_cur.append(v0)
            for t in range(T):
                for s in range(S):
                    e_t = ep.tile([NL, G], e_dt, tag=f"e{s}")
                    nc.sync.dma_start(e_t[:], e_ds[s][:, t * G:(t + 1) * G])
                    ps = pp.tile([NL, G], f32, tag=f"ps{s}")
                    nc.tensor.matmul(ps[:], w_t[:], v_cur[s][:],
                                     start=True, stop=True)
                    v_n = vp.tile([NL, G], f32, tag=f"v{s}")
                    nc.vector.tensor_tensor(v_n[:], ps[:], e_t[:],
                                            op=mybir.AluOpType.mult)
                    v_cur[s] = v_n
                    if t == W - 1:
                        nc.sync.dma_start(snap_ds[s][:], v_n[:])
                    if t == T - 1:
                        nc.sync.dma_start(fin_ds[s][:], v_n[:])
    nc.compile()
    nc.finalize()
    return nc


def _profiled_run(nc, in_maps):
    """Run via PJRT with local NTFF profiling (core 0); returns (res, ns)."""
    import tempfile
    from concourse.bass_utils import run_bass_kernel_spmd
    from trn_agent_boot.trn_boot import _ntff_profile_via_ctypes

    hookf = _ntff_profile_via_ctypes("/opt/axon/libaxon_pjrt.so")
    neff_dir = tempfile.mkdtemp(prefix="crfprof_")
    exec_ns = None
    if hookf is None:
        res = run_bass_kernel_spmd(nc, in_maps, list(range(N_CORES)))
        return res, None
    with hookf(neff_dir, [0]):
        res = run_bass_kernel_spmd(nc, in_maps, list(range(N_CORES)))
    try:
        import gauge.profiler
        from concourse._compat import FishPath

        profile = gauge.profiler.Profile(
            profile_path=FishPath(neff_dir),
            kernel_dev_mode=True,
            profile_on_exit=False,
            bass_kernel=nc.m,
            offline_processing=True,
            fname="*_body*",
        )
        results = profile.to_perfetto(model_index=(0,))
        if results:
            exec_ns = results[0].exec_time_ns
            print(f"[profile] core0 exec {exec_ns} ns, "
                  f"trace: {results[0].trace_path}")
    except Exception as e:  # profiling must never break the run
        import traceback
        traceback.print_exc()
        print(f"[profile] failed: {e!r} (dir {neff_dir})")
    return res, exec_ns


def _estimate_mu(obs_pad, expTz64, n=256, skip=32):
    """Mean per-step log growth of the linear-space recursion (fp64)."""
    u = np.full(NL, 1.0 / NL, np.float64)
    logs = []
    for t in range(skip + n):
        u = (u @ expTz64) * np.exp(obs_pad[t].astype(np.float64))
        ssum = u.sum()
        logs.append(np.log(ssum))
        u /= ssum
    return float(np.mean(logs[skip:]))


def kernel(pred: np.ndarray, transitions: np.ndarray, ref: np.ndarray) -> np.ndarray:
    global last_exec_time_ns
    pred = np.asarray(pred)
    transitions = np.asarray(transitions)
    ref = np.asarray(ref)
    assert pred.shape == (SEQ, L)

    T64 = transitions.astype(np.float64)
    expTz64 = np.exp(T64)
    expTz64[:, L:] = 0.0            # dummy labels carry no mass mid-sequence

    obs_pad = np.zeros((SEQ, NL), np.float32)
    obs_pad[:, :L] = pred

    mu = _estimate_mu(obs_pad, expTz64)

    # E128T[l, p] = exp(obs[p, l] - mu), transposed, with W left-pad columns
    # replicating position 0 (burn-in clamp for the first groups).
    E128 = np.exp(obs_pad - np.float32(mu))
    E128T = np.ascontiguousarray(E128.T)                  # [128, SEQ]
    EPAD = np.concatenate(
        [np.repeat(E128T[:, :1], W, axis=1), E128T], axis=1
    )                                                     # [128, W + SEQ]

    np_e_dt = np.float32
    if E_DT == "bfloat16":
        import ml_dtypes
        np_e_dt = ml_dtypes.bfloat16

    # Device emission tiles: e[c][s][l, t, g] = EPAD[l, A + t + g*CHUNK],
    # A = (c*S + s)*G*CHUNK  (EPAD coords already include the +W shift).
    itemsize = EPAD.strides[1]
    e_blocks = []
    for c in range(N_CORES):
        per_s = []
        for s in range(S):
            A = (c * S + s) * G * CHUNK
            blk = np.lib.stride_tricks.as_strided(
                EPAD[:, A:],
                shape=(NL, T, G),
                strides=(EPAD.strides[0], itemsize, CHUNK * itemsize),
            )
            per_s.append(
                np.ascontiguousarray(blk).astype(np_e_dt).reshape(NL, T * G)
            )
        e_blocks.append(per_s)

    wz32 = expTz64.astype(np.float32)
    nc = _build_program()

    from concourse.bass_utils import run_bass_kernel_spmd
    in_maps = [
        {"w": wz32, **{f"e{s}": e_blocks[c][s] for s in range(S)}}
        for c in range(N_CORES)
    ]
    if os.environ.get("CRF_TRACE"):
        res, last_exec_time_ns = _profiled_run(nc, in_maps)
    else:
        res = run_bass_kernel_spmd(nc, in_maps, list(range(N_CORES)))

    # --- host stitch (fp64) -------------------------------------------------
    snap_sums = np.empty(C, np.float64)
    fin_sums = np.empty(C, np.float64)
    d_last = None
    for c in range(N_CORES):
        for s in range(S):
            j0 = (c * S + s) * G
            snap = res.results[c][f"snap{s}"].astype(np.float64)
            fin = res.results[c][f"fin{s}"].astype(np.float64)
            snap_sums[j0:j0 + G] = snap.sum(axis=0)
            fin_sums[j0:j0 + G] = fin.sum(axis=0)
            if c == N_CORES - 1 and s == S - 1:
                d_last = fin[:, G - 1] / fin[:, G - 1].sum()

    # group 0 exactly on host: start from begin-state e_{126}
    u = np.zeros(NL, np.float64)
    u[L] = 1.0
    log_g0 = 0.0
    for t in range(CHUNK):
        u = (u @ expTz64) * np.exp(obs_pad[t].astype(np.float64))
        ssum = u.sum()
        log_g0 += np.log(ssum)
        u /= ssum

    ratios = np.log(fin_sums[1:]) - np.log(snap_sums[1:])
    logZ = (
        log_g0
        + ratios.sum() + (C - 1) * CHUNK * mu
        + np.log(np.dot(d_last, np.exp(T64[:, L + 1])))
    )

    # gold path score
    idx = np.arange(SEQ)
    real = pred.astype(np.float64)[idx, ref].sum()
    padded = np.concatenate(
        [np.array([L], ref.dtype), ref, np.array([L + 1], ref.dtype)]
    )
    real += T64[padded[:-1], padded[1:]].sum()

    return np.float32(logZ - real)


# revision 17
# speedup vs baseline: 1.5502x; 1.5502x over previous
"""CRF loss kernel for Trainium2 (8 NeuronCores).

Forward CRF recursion in linear space:
    alpha_t = (alpha_{t-1} @ expT) * exp(o_t)
Positive matrices forget initial conditions geometrically (Birkhoff
contraction; here expT entries are within e^{+-0.5} of 1, so the rate is
~5x per step). The sequence is therefore cut into C short chunks; each
chunk's chain is started from a uniform state W steps early (burn-in) so
its state direction converges to the true alpha before the chunk begins.
Per-group log-growth over the chunk is exact, directions are accurate to
~kappa^W; the scalar log-partition is stitched on the host in fp64.

Device layout: state V[128 labels (partitions), G groups (free dim)].
One matmul with the constant expT as stationary weights advances ALL G
groups one step; a DVE elementwise multiply applies the per-position
emission factors (host-gathered tiles). Emissions are pre-scaled by
exp(-mu) (mu = mean per-step log growth) so no renormalisation is ever
needed: fp32 state drift stays within e^{+-3}.

Group 0 has no real burn-in data (clamped), so its chunk is recomputed
exactly on the host in fp64 and its device result discarded.
"""

import os
import numpy as np

SEQ = 131072
L = 126          # real labels; transitions is (L+2, L+2) = (128, 128)
NL = 128
N_CORES = 8

# ---- tunables -------------------------------------------------------------
S = 2            # superchains (PSUM banks) per core
G = 512          # groups per superchain (one matmul/DVE op advances G chains)
W = 8            # burn-in steps
E_DT = "bfloat16"  # emission/state dtype on device: "float32" or "bfloat16"
# ---------------------------------------------------------------------------

C = N_CORES * S * G               # total groups
CHUNK = SEQ // C                  # steps per group
T = W + CHUNK                     # device steps per superchain
assert CHUNK * C == SEQ

last_exec_time_ns = None


def _build_program():
    import concourse.bacc as bacc
    import concourse.mybir as mybir
    from concourse.tile import TileContext

    e_dt = getattr(mybir.dt, E_DT)
    f32 = mybir.dt.float32

    nc = bacc.Bacc("TRN2", target_bir_lowering=False, debug=False)
    w_d = nc.dram_tensor("w", [NL, NL], e_dt, kind="ExternalInput")
    e_ds = [
        nc.dram_tensor(f"e{s}", [NL, T * G], e_dt, kind="ExternalInput")
        for s in range(S)
    ]
    snap_ds = [
        nc.dram_tensor(f"snap{s}", [NL, G], e_dt, kind="ExternalOutput")
        for s in range(S)
    ]
    fin_ds = [
        nc.dram_tensor(f"fin{s}", [NL, G], e_dt, kind="ExternalOutput")
        for s in range(S)
    ]

    with TileContext(nc) as tc:
        with tc.tile_pool(name="wp", bufs=1) as wp, \
             tc.tile_pool(name="ep", bufs=6) as ep, \
             tc.tile_pool(name="vp", bufs=3) as vp, \
             tc.tile_pool(name="pp", bufs=2, space="PSUM") as pp:
            # Stage weights through a DVE copy: the first matmul then waits
            # only on the DVE semaphore (PE LDW allows a single sync wait).
            w_stage = wp.tile([NL, NL], e_dt, tag="wstage")
            nc.sync.dma_start(w_stage[:], w_d[:])
            w_t = wp.tile([NL, NL], e_dt, tag="wt")
            nc.vector.tensor_copy(w_t[:], w_stage[:])
            v_cur = []
            for s in range(S):
                v0 = vp.tile([NL, G], e_dt, tag=f"v{s}")
                nc.vector.memset(v0[:], 1.0)
                v_cur.append(v0)
            for t in range(T):
                for s in range(S):
                    e_t = ep.tile([NL, G], e_dt, tag=f"e{s}")
                    nc.sync.dma_start(e_t[:], e_ds[s][:, t * G:(t + 1) * G])
                    ps = pp.tile([NL, G], f32, tag=f"ps{s}")
                    nc.tensor.matmul(ps[:], w_t[:], v_cur[s][:],
                                     start=True, stop=True)
                    v_n = vp.tile([NL, G], e_dt, tag=f"v{s}")
                    nc.vector.tensor_tensor(v_n[:], ps[:], e_t[:],
                                            op=mybir.AluOpType.mult)
                    v_cur[s] = v_n
                    if t == W - 1:
                        nc.sync.dma_start(snap_ds[s][:], v_n[:])
                    if t == T - 1:
                        nc.sync.dma_start(fin_ds[s][:], v_n[:])
    nc.compile()
    nc.finalize()
    return nc


def _profiled_run(nc, in_maps):
    """Run via PJRT with local NTFF profiling (core 0); returns (res, ns)."""
    import tempfile
    from concourse.bass_utils import run_bass_kernel_spmd
    from trn_agent_boot.trn_boot import _ntff_profile_via_ctypes

    hookf = _ntff_profile_via_ctypes("/opt/axon/libaxon_pjrt.so")
    neff_dir = tempfile.mkdtemp(prefix="crfprof_")
    exec_ns = None
    if hookf is None:
        res = run_bass_kernel_spmd(nc, in_maps, list(range(N_CORES)))
        return res, None
    with hookf(neff_dir, [0]):
        res = run_bass_kernel_spmd(nc, in_maps, list(range(N_CORES)))
    try:
        import gauge.profiler
        from concourse._compat import FishPath

        profile = gauge.profiler.Profile(
            profile_path=FishPath(neff_dir),
            kernel_dev_mode=True,
            profile_on_exit=False,
            bass_kernel=nc.m,
            offline_processing=True,
            fname="*_body*",
        )
        results = profile.to_perfetto(model_index=(0,))
        if results:
            exec_ns = results[0].exec_time_ns
            print(f"[profile] core0 exec {exec_ns} ns, "
                  f"trace: {results[0].trace_path}")
    except Exception as e:  # profiling must never break the run
        import traceback
        traceback.print_exc()
        print(f"[profile] failed: {e!r} (dir {neff_dir})")
    return res, exec_ns


def _estimate_mu(obs_pad, expTz64, n=256, skip=32):
    """Mean per-step log growth of the linear-space recursion (fp64)."""
    u = np.full(NL, 1.0 / NL, np.float64)
    logs = []
    for t in range(skip + n):
        u = (u @ expTz64) * np.exp(obs_pad[t].astype(np.float64))
        ssum = u.sum()
        logs.append(np.log(ssum))
        u /= ssum
    return float(np.mean(logs[skip:]))


def kernel(pred: np.ndarray, transitions: np.ndarray, ref: np.ndarray) -> np.ndarray:
    global last_exec_time_ns
    pred = np.asarray(pred)
    transitions = np.asarray(transitions)
    ref = np.asarray(ref)
    assert pred.shape == (SEQ, L)

    T64 = transitions.astype(np.float64)
    expTz64 = np.exp(T64)
    expTz64[:, L:] = 0.0            # dummy labels carry no mass mid-sequence

    obs_pad = np.zeros((SEQ, NL), np.float32)
    obs_pad[:, :L] = pred

    mu = _estimate_mu(obs_pad, expTz64)

    # E128T[l, p] = exp(obs[p, l] - mu), transposed, with W left-pad columns
    # replicating position 0 (burn-in clamp for the first groups).
    E128 = np.exp(obs_pad - np.float32(mu))
    E128T = np.ascontiguousarray(E128.T)                  # [128, SEQ]
    EPAD = np.concatenate(
        [np.repeat(E128T[:, :1], W, axis=1), E128T], axis=1
    )                                                     # [128, W + SEQ]

    np_e_dt = np.float32
    if E_DT == "bfloat16":
        import ml_dtypes
        np_e_dt = ml_dtypes.bfloat16
        EPAD = EPAD.astype(np_e_dt)

    # Device emission tiles: e[c][s][l, t, g] = EPAD[l, A + t + g*CHUNK],
    # A = (c*S + s)*G*CHUNK  (EPAD coords already include the +W shift).
    itemsize = EPAD.strides[1]
    e_blocks = []
    for c in range(N_CORES):
        per_s = []
        for s in range(S):
            A = (c * S + s) * G * CHUNK
            blk = np.lib.stride_tricks.as_strided(
                EPAD[:, A:],
                shape=(NL, T, G),
                strides=(EPAD.strides[0], itemsize, CHUNK * itemsize),
            )
            per_s.append(np.ascontiguousarray(blk).reshape(NL, T * G))
        e_blocks.append(per_s)

    wz32 = expTz64.astype(np_e_dt)
    nc = _build_program()

    from concourse.bass_utils import run_bass_kernel_spmd
    in_maps = [
        {"w": wz32, **{f"e{s}": e_blocks[c][s] for s in range(S)}}
        for c in range(N_CORES)
    ]
    if os.environ.get("CRF_TRACE"):
        res, last_exec_time_ns = _profiled_run(nc, in_maps)
    else:
        res = run_bass_kernel_spmd(nc, in_maps, list(range(N_CORES)))

    # --- host stitch (fp64) -------------------------------------------------
    snap_sums = np.empty(C, np.float64)
    fin_sums = np.empty(C, np.float64)
    d_last = None
    for c in range(N_CORES):
        for s in range(S):
            j0 = (c * S + s) * G
            snap = res.results[c][f"snap{s}"].astype(np.float64)
            fin = res.results[c][f"fin{s}"].astype(np.float64)
            snap_sums[j0:j0 + G] = snap.sum(axis=0)
            fin_sums[j0:j0 + G] = fin.sum(axis=0)
            if c == N_CORES - 1 and s == S - 1:
                d_last = fin[:, G - 1] / fin[:, G - 1].sum()

    # group 0 exactly on host: start from begin-state e_{126}
    u = np.zeros(NL, np.float64)
    u[L] = 1.0
    log_g0 = 0.0
    for t in range(CHUNK):
        u = (u @ expTz64) * np.exp(obs_pad[t].astype(np.float64))
        ssum = u.sum()
        log_g0 += np.log(ssum)
        u /= ssum

    ratios = np.log(fin_sums[1:]) - np.log(snap_sums[1:])
    logZ = (
        log_g0
        + ratios.sum() + (C - 1) * CHUNK * mu
        + np.log(np.dot(d_last, np.exp(T64[:, L + 1])))
    )

    # gold path score
    idx = np.arange(SEQ)
    real = pred.astype(np.float64)[idx, ref].sum()
    padded = np.concatenate(
        [np.array([L], ref.dtype), ref, np.array([L + 1], ref.dtype)]
    )
    real += T64[padded[:-1], padded[1:]].sum()

    return np.float32(logZ - real)
